# revision 7
# baseline (speedup 1.0000x reference)
"""Channel-attention (bmm-softmax-bmm over channels) on 8 TRN2 NeuronCores.

Math (per batch b, x: [C, P]):
    q = Wq x + bq 1^T ; k = Wk x + bk 1^T ; v = Wv x + bv 1^T
    E = q k^T ; attn = softmax(E, axis=-1) ; out = attn v

Gram reformulation with host-precomputed bias vectors:
    G  = x x^T                      (device, symmetric: upper block-row only)
    s  = x @ 1_P                    (host)
    qs = Wq s ; r = Wk s + P bk     (host)
    U  = G Wk^T                     (device)
    E  = Wq U + qs bk^T + bq r^T    (device; rank-2 term as one K=2 matmul)
    attn_un = exp(E - rowmax), Z = rowsum  (softmax read directly from PSUM)
    AT = Wv^T attn_un^T ; rt = (attn_un @ bv)/Z
    out = (AT^T x) * (1/Z) + rt 1^T

v4: the whole pipeline runs in FP16.  On TRN2, fp32r matmuls stream at ~2
PE-cycles per column (fp32_mode=HIGH, SBUF-bandwidth-bound) while 16-bit
matmuls stream at 1 — and fp16's 10-bit mantissa matches tf32 precision, so
fp16 is strictly better here than f32r (CPU-sim rel err 3.3e-3, fp32r 4.5e-3).
All matmul operands (xt, xb, weights, G, U, attn, AT) are fp16; PSUM
accumulation stays fp32.  x arrives both pre-transposed (xt) and row-major
(xb) from the host, so the PE does no x transposes.  G's four accumulator
block-rows are packed into 3 PSUM banks (512 | 384+128 | 256) which frees a
bank for triple-buffered out-phase PSUM.  A short burst of throwaway matmuls
on a memset tile warms the PE HAM clock gate during the DMA preamble.
Output stores go out one-per-quarter on the gpsimd (SWDGE) queue; the final
quarters fan out per-c-tile across the sync+gpsimd queues so the tail drain
is short.  Sharding: data-parallel over B, core i gets batches [2i, 2i+1];
no cross-core comms.
"""

from contextlib import ExitStack

import numpy as np
import ml_dtypes

import concourse.bass as bass  # noqa: F401
from concourse import bacc
import concourse.mybir as mybir
import concourse.tile as tile
from concourse.bass_utils import run_bass_kernel_spmd

B, C, P = 16, 512, 4096
N_CORES = 8
BPC = B // N_CORES           # batches per core
CT = C // 128                # 4 c-tiles
QTR = P // 4                 # 1024-wide p quarters
NQ = 4                       # quarters per batch
QT_Q = QTR // 128            # 8 p-tiles per quarter
F32 = mybir.dt.float32
FP16 = mybir.dt.float16
BF16 = mybir.dt.bfloat16

AX = mybir.AxisListType
ALU = mybir.AluOpType
ACTF = mybir.ActivationFunctionType

N_WARMUP = 8                 # ~3.4us of throwaway matmuls flips HAM to 8/8


def build_nc():
    ST = FP16
    nc = bacc.Bacc(trn_type="TRN2", target_bir_lowering=False, debug=False)

    xt_d = nc.dram_tensor("xt", [BPC, P, C], ST, kind="ExternalInput")
    xb_d = nc.dram_tensor("xb", [BPC, C, P], BF16, kind="ExternalInput")
    wqt_d = nc.dram_tensor("wqt", [C, C], ST, kind="ExternalInput")
    wkt_d = nc.dram_tensor("wkt", [C, C], ST, kind="ExternalInput")
    wv_d = nc.dram_tensor("wv", [C, C], BF16, kind="ExternalInput")
    l2_d = nc.dram_tensor("l2", [BPC, 2, C], ST, kind="ExternalInput")
    r2_d = nc.dram_tensor("r2", [BPC, 2, C], ST, kind="ExternalInput")
    bvr_d = nc.dram_tensor("bv_row", [1, C], F32, kind="ExternalInput")
    ident_d = nc.dram_tensor("ident", [128, 128], ST, kind="ExternalInput")
    out_d = nc.dram_tensor("out", [BPC, C, P], BF16, kind="ExternalOutput")

    with ExitStack() as ctx:
        tc = ctx.enter_context(tile.TileContext(nc))
        const = ctx.enter_context(tc.tile_pool(name="const", bufs=1))
        xtp = ctx.enter_context(tc.tile_pool(name="xtp", bufs=3))
        xbp = ctx.enter_context(tc.tile_pool(name="xbp", bufs=4))
        midp = ctx.enter_context(tc.tile_pool(name="midp", bufs=2))
        vecp = ctx.enter_context(tc.tile_pool(name="vecp", bufs=2))
        outp = ctx.enter_context(tc.tile_pool(name="outp", bufs=3))
        gps = ctx.enter_context(tc.tile_pool(name="gps", bufs=1, space="PSUM"))
        mmps = ctx.enter_context(tc.tile_pool(name="mmps", bufs=2, space="PSUM"))
        ops = ctx.enter_context(tc.tile_pool(name="ops", bufs=3, space="PSUM"))

        st0, st1 = {}, {}

        # ---- HAM warmup: no DMA dependency (memset tile), so it runs
        # during the fixed framework preamble + first-load latency ----
        warm = const.tile([128, 512], ST, name="warm")
        nc.vector.memset(warm, 0.5)
        warm_ps = mmps.tile([128, 512], F32, name="warm_ps", tag="mm")
        for _ in range(N_WARMUP):
            nc.tensor.matmul(
                out=warm_ps, lhsT=warm[:, 0:128], rhs=warm, start=True, stop=True
            )

        # ---- small consts ride the scalar (HWDGE #2) queue so the sync
        # queue is exclusively the big xt/xb streams ----
        ident = const.tile([128, 128], ST, name="ident")
        nc.scalar.dma_start(out=ident, in_=ident_d[:, :])

        def load_l2r2(b, st):
            l2 = vecp.tile([2, C], ST, name=f"l2_b{b}", tag="l2")
            nc.scalar.dma_start(out=l2, in_=l2_d[b])
            r2 = vecp.tile([2, C], ST, name=f"r2_b{b}", tag="r2")
            nc.scalar.dma_start(out=r2, in_=r2_d[b])
            st["l2"], st["r2"] = l2, r2

        load_l2r2(0, st0)
        bv_rep = const.tile([128, C], F32, name="bv_rep")
        nc.scalar.dma_start(out=bv_rep, in_=bvr_d[:, :].partition_broadcast(128))
        load_l2r2(1, st1)

        # ---- big loads (sync queue FIFO = transfer order) ----
        def load_xt(b, q, st, split=1, eng=None):
            t = xtp.tile([128, QT_Q, C], ST, name=f"xt_b{b}q{q}", tag="xt")
            w = QT_Q // split
            for s_ in range(split):
                (eng or nc.sync).dma_start(
                    out=t[:, s_ * w : (s_ + 1) * w, :],
                    in_=xt_d[
                        b, q * QTR + s_ * w * 128 : q * QTR + (s_ + 1) * w * 128, :
                    ].rearrange("(t p) c -> p t c", p=128),
                )
            st[f"xt{q}"] = t

        def load_xb(b, q, st):
            t = xbp.tile([128, CT, QTR], BF16, name=f"xb_b{b}q{q}", tag="xb")
            nc.sync.dma_start(
                out=t,
                in_=xb_d[b, :, q * QTR : (q + 1) * QTR].rearrange(
                    "(t p) f -> p t f", p=128
                ),
            )
            st[f"xb{q}"] = t

        def load_w(name, d, dt=ST):
            t = const.tile([128, CT, C], dt, name=name)
            nc.sync.dma_start(out=t, in_=d[:, :].rearrange("(t p) f -> p t f", p=128))
            return t

        load_xt(0, 0, st0, split=2)
        load_xt(0, 1, st0, split=2, eng=nc.scalar)
        load_xt(0, 2, st0)
        load_xt(0, 3, st0, eng=nc.scalar)
        wkt_sb = load_w("wkt_sb", wkt_d)   # needed first (U phase)
        wqt_sb = load_w("wqt_sb", wqt_d)

        def copy_evac(i, out, in_):
            # alternate evacuation engine to balance DVE/ACT load
            if i % 2 == 0:
                nc.scalar.copy(out, in_)
            else:
                nc.vector.tensor_copy(out, in_)

        # ---- per-batch phases ----
        # G's 4 accumulator block-rows (widths 512/384/256/128 fp32) pack
        # into 3 PSUM banks: g0=cc0, g1=cc1(cols 0:384)+cc3(cols 384:512),
        # g2=cc2.  Bank-level start on the bank's first matmul; the second
        # group's first write lands on still-pending-zero bytes and
        # overwrites, which is exactly first-write semantics.
        def G_ptiles(b, st, q, ks):
            if "G_ps" not in st:
                g0 = gps.tile([128, 512], F32, name=f"g0_b{b}", tag="g0")
                g1 = gps.tile([128, 512], F32, name=f"g1_b{b}", tag="g1")
                g2 = gps.tile([128, 256], F32, name=f"g2_b{b}", tag="g2")
                st["G_ps"] = (g0, g1, g2)
            g0, g1, g2 = st["G_ps"]
            targets = [
                (g0, 0, 512),
                (g1, 0, 384),
                (g2, 0, 256),
                (g1, 384, 128),
            ]
            xt = st[f"xt{q}"]
            for k in ks:
                first = q == 0 and k == 0
                last = q == NQ - 1 and k == QT_Q - 1
                for cc, (tgt, off, w) in enumerate(targets):
                    nc.tensor.matmul(
                        out=tgt[:, off : off + w],
                        lhsT=xt[:, k, cc * 128 : (cc + 1) * 128],
                        rhs=xt[:, k, cc * 128 :],
                        start=first and cc < 3,
                        stop=last and cc != 1,
                    )

        def G_copy(b, st):
            """Evacuate the upper-triangle block-row of G and mirror the
            strictly-lower blocks via PE transposes (G is symmetric)."""
            g0, g1, g2 = st["G_ps"]
            G_sb = midp.tile([128, CT, C], ST, name="G_sb", tag="gsb")
            copy_evac(0, G_sb[:, 0, 0:512], g0)
            copy_evac(1, G_sb[:, 1, 128:512], g1[:, 0:384])
            copy_evac(2, G_sb[:, 2, 256:512], g2)
            copy_evac(3, G_sb[:, 3, 384:512], g1[:, 384:512])
            pairs = [(dd, cc) for cc in range(CT) for dd in range(cc)]
            lps = [mmps.tile([128, C], ST, name="lps", tag="mm") for _ in range(2)]
            for i, (dd, cc) in enumerate(pairs):
                nc.tensor.transpose(
                    out=lps[i // 4][:, (i % 4) * 128 : (i % 4 + 1) * 128],
                    in_=G_sb[:, dd, cc * 128 : (cc + 1) * 128],
                    identity=ident,
                )
            for i, (dd, cc) in enumerate(pairs):
                copy_evac(
                    i,
                    G_sb[:, cc, dd * 128 : (dd + 1) * 128],
                    lps[i // 4][:, (i % 4) * 128 : (i % 4 + 1) * 128],
                )
            st["G_sb"] = G_sb
            del st["G_ps"]

        def U_phase(b, st):
            U_sb = midp.tile([128, CT, C], ST, name="U_sb", tag="usb")
            for ic in range(CT):
                u_ps = ops.tile([128, C], F32, name="u_ps", tag="out")
                for e in range(CT):
                    nc.tensor.matmul(
                        out=u_ps,
                        lhsT=st["G_sb"][:, e, ic * 128 : (ic + 1) * 128],
                        rhs=wkt_sb[:, e, :],
                        start=(e == 0),
                        stop=(e == CT - 1),
                    )
                copy_evac(ic, U_sb[:, ic, :], u_ps)
            st["U_sb"] = U_sb

        def E_prep(b, st):
            st["attn"] = midp.tile([128, CT, C], ST, name="attn_sb", tag="attn")
            st["mx"] = vecp.tile([128, CT], F32, name="mx", tag="mx")
            st["negm"] = vecp.tile([128, CT], F32, name="negm", tag="negm")
            st["zsum"] = vecp.tile([128, CT], F32, name="zsum", tag="zsum")

        def E_cc(b, cc, st):
            # E block-row cc: 4 Gram matmuls + one K=2 rank-2 bias matmul;
            # softmax (max, exp+rowsum) reads the PSUM bank directly.  The
            # e_ps bank lives on the mm ring so out-phase matmuls on the
            # out ring never stall behind softmax reads.
            e_ps = mmps.tile([128, C], F32, name="e_ps", tag="mm")
            for i in range(CT):
                nc.tensor.matmul(
                    out=e_ps,
                    lhsT=wqt_sb[:, i, cc * 128 : (cc + 1) * 128],
                    rhs=st["U_sb"][:, i, :],
                    start=(i == 0),
                    stop=False,
                )
            nc.tensor.matmul(
                out=e_ps,
                lhsT=st["l2"][:, cc * 128 : (cc + 1) * 128],
                rhs=st["r2"],
                start=False,
                stop=True,
            )
            nc.vector.reduce_max(
                out=st["mx"][:, cc : cc + 1], in_=e_ps, axis=AX.X
            )
            nc.vector.tensor_scalar_mul(
                st["negm"][:, cc : cc + 1], st["mx"][:, cc : cc + 1], -1.0
            )
            nc.scalar.activation(
                out=st["attn"][:, cc, :],
                in_=e_ps,
                func=ACTF.Exp,
                bias=st["negm"][:, cc : cc + 1],
                scale=1.0,
                accum_out=st["zsum"][:, cc : cc + 1],
            )

        def finish_softmax(b, st):
            recip = vecp.tile([128, CT], F32, name="recip", tag="recip")
            nc.vector.reciprocal(out=recip, in_=st["zsum"])
            tts = vecp.tile([128, C], F32, name="tts", tag="tts", bufs=1)
            tcol = vecp.tile([128, CT], F32, name="tcol", tag="tcol")
            for cc in range(CT):
                nc.vector.tensor_mul(tts, st["attn"][:, cc, :], bv_rep)
                nc.vector.reduce_sum(out=tcol[:, cc : cc + 1], in_=tts, axis=AX.X)
            rt = vecp.tile([128, CT], F32, name="rt", tag="rt")
            nc.vector.tensor_mul(rt, tcol, recip)
            st["recip"] = recip
            st["rt"] = rt

        def attnT_AT(b, st):
            attnT_sb = midp.tile([128, CT, C], BF16, name="attnT_sb", tag="attnT")
            for dc in range(CT):
                at_ps = mmps.tile([128, C], ST, name="at_ps", tag="mm")
                for t in range(CT):
                    nc.tensor.transpose(
                        out=at_ps[:, t * 128 : (t + 1) * 128],
                        in_=st["attn"][:, t, dc * 128 : (dc + 1) * 128],
                        identity=ident,
                    )
                copy_evac(dc, attnT_sb[:, dc, :], at_ps)
            AT_sb = midp.tile([128, CT, C], BF16, name="AT_sb", tag="atb")
            for ic in range(CT):
                a_ps = mmps.tile([128, C], F32, name="a_ps", tag="mm")
                for d_ in range(CT):
                    nc.tensor.matmul(
                        out=a_ps,
                        lhsT=wv_sb[:, d_, ic * 128 : (ic + 1) * 128],
                        rhs=attnT_sb[:, d_, :],
                        start=(d_ == 0),
                        stop=(d_ == CT - 1),
                    )
                copy_evac(ic + 1, AT_sb[:, ic, :], a_ps)
            st["AT"] = AT_sb

        def out_q_begin(b, q, st):
            st[f"stage{q}"] = outp.tile(
                [128, CT, QTR], BF16, name=f"stage_b{b}q{q}", tag="stage"
            )

        def out_cc(b, q, cc, st, use_gps=False):
            xb = st[f"xb{q}"]
            stage = st[f"stage{q}"]
            for pb in range(2):
                # late quarters rotate through the dead G-accumulator banks
                # too (5-deep ring) so the PSUM-evacuation affine latency
                # never gates the next matmul group
                g = st.get("ogrp", 0)
                st["ogrp"] = g + 1
                if use_gps and g % 5 == 3:
                    o_ps = gps.tile([128, 512], F32, name="o_ps_g0", tag="g0")
                elif use_gps and g % 5 == 4:
                    o_ps = gps.tile([128, 512], F32, name="o_ps_g1", tag="g1")
                else:
                    o_ps = ops.tile([128, 512], F32, name="o_ps", tag="out")
                for i in range(CT):
                    nc.tensor.matmul(
                        out=o_ps,
                        lhsT=st["AT"][:, i, cc * 128 : (cc + 1) * 128],
                        rhs=xb[:, i, pb * 512 : (pb + 1) * 512],
                        start=(i == 0),
                        stop=(i == CT - 1),
                    )
                if pb % 2 == 0:
                    nc.scalar.activation(
                        out=stage[:, cc, pb * 512 : (pb + 1) * 512],
                        in_=o_ps,
                        func=ACTF.Identity,
                        bias=st["rt"][:, cc : cc + 1],
                        scale=st["recip"][:, cc : cc + 1],
                    )
                else:
                    nc.vector.tensor_scalar(
                        out=stage[:, cc, pb * 512 : (pb + 1) * 512],
                        in0=o_ps,
                        scalar1=st["recip"][:, cc : cc + 1],
                        scalar2=st["rt"][:, cc : cc + 1],
                        op0=ALU.mult,
                        op1=ALU.add,
                    )

        def out_q_store(b, q, st, split=False):
            stage = st.pop(f"stage{q}")
            if not split:
                # one store per quarter on the gpsimd (SWDGE) queue: few
                # queue ops, never blocks the load queues
                nc.gpsimd.dma_start(
                    out=out_d[b, :, q * QTR : (q + 1) * QTR].rearrange(
                        "(t p) f -> p t f", p=128
                    ),
                    in_=stage,
                )
            else:
                # tail quarters: fan out per c-tile across idle queues so
                # the final drain is short
                for cc in range(CT):
                    eng = nc.sync if cc % 2 == 0 else nc.scalar
                    eng.dma_start(
                        out=out_d[
                            b, cc * 128 : (cc + 1) * 128, q * QTR : (q + 1) * QTR
                        ],
                        in_=stage[:, cc, :],
                    )

        def out_phase(b, q, st, split=False, use_gps=False):
            out_q_begin(b, q, st)
            for cc in range(CT):
                out_cc(b, q, cc, st, use_gps=use_gps)
            out_q_store(b, q, st, split=split)

        # ---- schedule: batch-1 G matmuls are threaded into batch-0's
        # softmax/evac latency windows (and b0's last out quarter into
        # b1's) so the PE never idles long enough to re-throttle ----
        for q in range(NQ):
            G_ptiles(0, st0, q, range(QT_Q))
        G_copy(0, st0)
        wv_sb = load_w("wv_sb", wv_d, BF16)
        load_xt(1, 0, st1)
        U_phase(0, st0)
        E_prep(0, st0)
        E_cc(0, 0, st0)
        E_cc(0, 1, st0)
        load_xt(1, 1, st1)
        G_ptiles(1, st1, 0, range(0, 4))
        E_cc(0, 2, st0)
        G_ptiles(1, st1, 0, range(4, 8))
        E_cc(0, 3, st0)
        load_xb(0, 0, st0)
        G_ptiles(1, st1, 1, range(0, 4))
        finish_softmax(0, st0)
        attnT_AT(0, st0)
        G_ptiles(1, st1, 1, range(4, 8))
        load_xb(0, 1, st0)
        load_xt(1, 2, st1)
        out_phase(0, 0, st0)
        G_ptiles(1, st1, 2, range(0, 8))
        load_xt(1, 3, st1)
        load_xb(0, 2, st0)
        out_phase(0, 1, st0)
        G_ptiles(1, st1, 3, range(0, 8))
        load_xb(0, 3, st0)
        out_phase(0, 2, st0)
        G_copy(1, st1)
        load_xb(1, 0, st1)
        U_phase(1, st1)
        E_prep(1, st1)
        out_q_begin(0, 3, st0)
        E_cc(1, 0, st1)
        out_cc(0, 3, 0, st0, use_gps=True)
        E_cc(1, 1, st1)
        out_cc(0, 3, 1, st0, use_gps=True)
        load_xb(1, 1, st1)
        E_cc(1, 2, st1)
        out_cc(0, 3, 2, st0, use_gps=True)
        E_cc(1, 3, st1)
        out_cc(0, 3, 3, st0, use_gps=True)
        out_q_store(0, 3, st0)
        load_xb(1, 2, st1)
        finish_softmax(1, st1)
        attnT_AT(1, st1)
        load_xb(1, 3, st1)
        out_phase(1, 0, st1, use_gps=True)
        out_phase(1, 1, st1, use_gps=True)
        out_phase(1, 2, st1, split=True, use_gps=True)
        out_phase(1, 3, st1, split=True, use_gps=True)

    nc.compile()
    return nc


_CACHE = {}


def _get_nc():
    if "nc" not in _CACHE:
        _CACHE["nc"] = build_nc()
    return _CACHE["nc"]


def make_in_maps(x, Wq, bq, Wk, bk, Wv, bv):
    x = np.asarray(x, np.float32)
    Wq = np.asarray(Wq, np.float32)
    Wk = np.asarray(Wk, np.float32)
    Wv = np.asarray(Wv, np.float32)
    bq = np.asarray(bq, np.float32)
    bk = np.asarray(bk, np.float32)
    bv = np.asarray(bv, np.float32)
    f16 = np.float16
    shared = {
        "wqt": np.ascontiguousarray(Wq.T.astype(f16)),
        "wkt": np.ascontiguousarray(Wk.T.astype(f16)),
        "wv": np.ascontiguousarray(Wv.astype(ml_dtypes.bfloat16)),
        "bv_row": np.ascontiguousarray(bv[None, :]),
        "ident": np.eye(128, dtype=f16),
    }
    maps = []
    for i in range(N_CORES):
        xs = x[BPC * i : BPC * (i + 1)]                    # [BPC, C, P]
        s = xs.sum(axis=2, dtype=np.float64).astype(np.float32)
        qs = s @ Wq.T
        r = s @ Wk.T + np.float32(P) * bk[None, :]
        l2 = np.stack([qs, np.broadcast_to(bq, (BPC, C))], axis=1)
        r2 = np.stack([np.broadcast_to(bk, (BPC, C)), r], axis=1)
        maps.append(
            {
                "xt": np.ascontiguousarray(xs.transpose(0, 2, 1).astype(f16)),
                "xb": np.ascontiguousarray(xs.astype(ml_dtypes.bfloat16)),
                "l2": np.ascontiguousarray(l2.astype(f16)),
                "r2": np.ascontiguousarray(r2.astype(f16)),
                **shared,
            }
        )
    return maps


def run(inputs, trace=False, tmpdir=None):
    nc = _get_nc()
    in_maps = make_in_maps(**inputs)
    res = run_bass_kernel_spmd(
        nc, in_maps, core_ids=list(range(N_CORES)), trace=trace, tmpdir=tmpdir
    )
    out = np.concatenate(
        [res.results[i]["out"].astype(np.float32) for i in range(N_CORES)], axis=0
    )
    return out, res


def kernel(**inputs) -> np.ndarray:
    out, _ = run(inputs, trace=False)
    return out


# revision 8
# speedup vs baseline: 1.0594x; 1.0594x over previous
"""Channel-attention (bmm-softmax-bmm over channels) on 8 TRN2 NeuronCores.

Math (per batch b, x: [C, P]):
    q = Wq x + bq 1^T ; k = Wk x + bk 1^T ; v = Wv x + bv 1^T
    E = q k^T ; attn = softmax(E, axis=-1) ; out = attn v

Gram reformulation with host-precomputed bias vectors:
    G  = x x^T                      (device, symmetric: upper block-row only)
    s  = x @ 1_P                    (host)
    qs = Wq s ; r = Wk s + P bk     (host)
    U  = G Wk^T                     (device)
    E  = Wq U + qs bk^T + bq r^T    (device; rank-2 term as one K=2 matmul)
    attn_un = exp(E - rowmax), Z = rowsum  (softmax read directly from PSUM)
    AT = Wv^T attn_un^T ; rt = (attn_un @ bv)/Z
    out = (AT^T x) * (1/Z) + rt 1^T

v4: the whole pipeline runs in FP16.  On TRN2, fp32r matmuls stream at ~2
PE-cycles per column (fp32_mode=HIGH, SBUF-bandwidth-bound) while 16-bit
matmuls stream at 1 — and fp16's 10-bit mantissa matches tf32 precision, so
fp16 is strictly better here than f32r (CPU-sim rel err 3.3e-3, fp32r 4.5e-3).
All matmul operands (xt, xb, weights, G, U, attn, AT) are fp16; PSUM
accumulation stays fp32.  x arrives both pre-transposed (xt) and row-major
(xb) from the host, so the PE does no x transposes.  G's four accumulator
block-rows are packed into 3 PSUM banks (512 | 384+128 | 256) which frees a
bank for triple-buffered out-phase PSUM.  A short burst of throwaway matmuls
on a memset tile warms the PE HAM clock gate during the DMA preamble.
Output stores go out one-per-quarter on the gpsimd (SWDGE) queue; the final
quarters fan out per-c-tile across the sync+gpsimd queues so the tail drain
is short.  Sharding: data-parallel over B, core i gets batches [2i, 2i+1];
no cross-core comms.
"""

from contextlib import ExitStack

import numpy as np
import ml_dtypes

import concourse.bass as bass  # noqa: F401
from concourse import bacc
import concourse.mybir as mybir
import concourse.tile as tile
from concourse.bass_utils import run_bass_kernel_spmd

B, C, P = 16, 512, 4096
N_CORES = 8
BPC = B // N_CORES           # batches per core
CT = C // 128                # 4 c-tiles
QTR = P // 4                 # 1024-wide p quarters
NQ = 4                       # quarters per batch
QT_Q = QTR // 128            # 8 p-tiles per quarter
F32 = mybir.dt.float32
FP16 = mybir.dt.float16
BF16 = mybir.dt.bfloat16

AX = mybir.AxisListType
ALU = mybir.AluOpType
ACTF = mybir.ActivationFunctionType

N_WARMUP = 8                 # ~3.4us of throwaway matmuls flips HAM to 8/8


def build_nc():
    ST = FP16
    nc = bacc.Bacc(trn_type="TRN2", target_bir_lowering=False, debug=False)

    xt_d = nc.dram_tensor("xt", [BPC, P // 256, 128, 2 * C], ST, kind="ExternalInput")
    xb_d = nc.dram_tensor("xb", [BPC, C, P], BF16, kind="ExternalInput")
    wqt_d = nc.dram_tensor("wqt", [C, C], ST, kind="ExternalInput")
    wkt_d = nc.dram_tensor("wkt", [C, C], ST, kind="ExternalInput")
    wv_d = nc.dram_tensor("wv", [C, C], BF16, kind="ExternalInput")
    l2_d = nc.dram_tensor("l2", [BPC, 2, C], ST, kind="ExternalInput")
    r2_d = nc.dram_tensor("r2", [BPC, 2, C], ST, kind="ExternalInput")
    bvr_d = nc.dram_tensor("bv_row", [1, C], F32, kind="ExternalInput")
    ident_d = nc.dram_tensor("ident", [128, 128], ST, kind="ExternalInput")
    out_d = nc.dram_tensor("out", [BPC, C, P], BF16, kind="ExternalOutput")

    with ExitStack() as ctx:
        tc = ctx.enter_context(tile.TileContext(nc))
        const = ctx.enter_context(tc.tile_pool(name="const", bufs=1))
        xtp = ctx.enter_context(tc.tile_pool(name="xtp", bufs=3))
        xbp = ctx.enter_context(tc.tile_pool(name="xbp", bufs=4))
        midp = ctx.enter_context(tc.tile_pool(name="midp", bufs=2))
        vecp = ctx.enter_context(tc.tile_pool(name="vecp", bufs=2))
        outp = ctx.enter_context(tc.tile_pool(name="outp", bufs=3))
        gps = ctx.enter_context(tc.tile_pool(name="gps", bufs=1, space="PSUM"))
        mmps = ctx.enter_context(tc.tile_pool(name="mmps", bufs=2, space="PSUM"))
        ops = ctx.enter_context(tc.tile_pool(name="ops", bufs=3, space="PSUM"))

        st0, st1 = {}, {}

        # ---- HAM warmup: no DMA dependency (memset tile), so it runs
        # during the fixed framework preamble + first-load latency ----
        warm = const.tile([128, 512], ST, name="warm")
        nc.vector.memset(warm, 0.5)
        warm_ps = mmps.tile([128, 512], F32, name="warm_ps", tag="mm")
        for _ in range(N_WARMUP):
            nc.tensor.matmul(
                out=warm_ps, lhsT=warm[:, 0:128], rhs=warm, start=True, stop=True
            )

        # ---- small consts ride the scalar (HWDGE #2) queue so the sync
        # queue is exclusively the big xt/xb streams ----
        ident = const.tile([128, 128], ST, name="ident")
        nc.scalar.dma_start(out=ident, in_=ident_d[:, :])

        def load_l2r2(b, st):
            l2 = vecp.tile([2, C], ST, name=f"l2_b{b}", tag="l2")
            nc.scalar.dma_start(out=l2, in_=l2_d[b])
            r2 = vecp.tile([2, C], ST, name=f"r2_b{b}", tag="r2")
            nc.scalar.dma_start(out=r2, in_=r2_d[b])
            st["l2"], st["r2"] = l2, r2

        load_l2r2(0, st0)
        bv_rep = const.tile([128, C], F32, name="bv_rep")
        nc.scalar.dma_start(out=bv_rep, in_=bvr_d[:, :].partition_broadcast(128))
        load_l2r2(1, st1)

        # ---- big loads (sync queue FIFO = transfer order) ----
        def load_xt(b, q, st, split=1, eng=None):
            # host packs p-tile pairs so each partition line is one 2KB
            # contiguous DMA chunk (1KB lines run ~25% under line rate)
            t = xtp.tile([128, QT_Q // 2, 2 * C], ST, name=f"xt_b{b}q{q}", tag="xt")
            w = QT_Q // 2 // split
            for s_ in range(split):
                (eng or nc.sync).dma_start(
                    out=t[:, s_ * w : (s_ + 1) * w, :],
                    in_=xt_d[
                        b, q * (QT_Q // 2) + s_ * w : q * (QT_Q // 2) + (s_ + 1) * w
                    ].rearrange("t p c -> p t c"),
                )
            st[f"xt{q}"] = t

        def load_xb(b, q, st):
            t = xbp.tile([128, CT, QTR], BF16, name=f"xb_b{b}q{q}", tag="xb")
            nc.sync.dma_start(
                out=t,
                in_=xb_d[b, :, q * QTR : (q + 1) * QTR].rearrange(
                    "(t p) f -> p t f", p=128
                ),
            )
            st[f"xb{q}"] = t

        def load_w(name, d, dt=ST):
            t = const.tile([128, CT, C], dt, name=name)
            nc.sync.dma_start(out=t, in_=d[:, :].rearrange("(t p) f -> p t f", p=128))
            return t

        load_xt(0, 0, st0, split=2)
        load_xt(0, 1, st0)
        load_xt(0, 2, st0)
        load_xt(0, 3, st0)
        wkt_sb = load_w("wkt_sb", wkt_d)   # needed first (U phase)
        wqt_sb = load_w("wqt_sb", wqt_d)

        def copy_evac(i, out, in_):
            # alternate evacuation engine to balance DVE/ACT load
            if i % 2 == 0:
                nc.scalar.copy(out, in_)
            else:
                nc.vector.tensor_copy(out, in_)

        # ---- per-batch phases ----
        # G's 4 accumulator block-rows (widths 512/384/256/128 fp32) pack
        # into 3 PSUM banks: g0=cc0, g1=cc1(cols 0:384)+cc3(cols 384:512),
        # g2=cc2.  Bank-level start on the bank's first matmul; the second
        # group's first write lands on still-pending-zero bytes and
        # overwrites, which is exactly first-write semantics.
        def G_ptiles(b, st, q, ks):
            if "G_ps" not in st:
                g0 = gps.tile([128, 512], F32, name=f"g0_b{b}", tag="g0")
                g1 = gps.tile([128, 512], F32, name=f"g1_b{b}", tag="g1")
                g2 = gps.tile([128, 256], F32, name=f"g2_b{b}", tag="g2")
                st["G_ps"] = (g0, g1, g2)
            g0, g1, g2 = st["G_ps"]
            targets = [
                (g0, 0, 512),
                (g1, 0, 384),
                (g2, 0, 256),
                (g1, 384, 128),
            ]
            xt = st[f"xt{q}"]
            for k in ks:
                first = q == 0 and k == 0
                last = q == NQ - 1 and k == QT_Q - 1
                base = (k % 2) * C
                for cc, (tgt, off, w) in enumerate(targets):
                    nc.tensor.matmul(
                        out=tgt[:, off : off + w],
                        lhsT=xt[:, k // 2, base + cc * 128 : base + (cc + 1) * 128],
                        rhs=xt[:, k // 2, base + cc * 128 : base + C],
                        start=first and cc < 3,
                        stop=last and cc != 1,
                    )

        def G_copy(b, st):
            """Evacuate the upper-triangle block-row of G and mirror the
            strictly-lower blocks via PE transposes (G is symmetric)."""
            g0, g1, g2 = st["G_ps"]
            G_sb = midp.tile([128, CT, C], ST, name="G_sb", tag="gsb")
            copy_evac(0, G_sb[:, 0, 0:512], g0)
            copy_evac(1, G_sb[:, 1, 128:512], g1[:, 0:384])
            copy_evac(2, G_sb[:, 2, 256:512], g2)
            copy_evac(3, G_sb[:, 3, 384:512], g1[:, 384:512])
            pairs = [(dd, cc) for cc in range(CT) for dd in range(cc)]
            lps = [mmps.tile([128, C], ST, name="lps", tag="mm") for _ in range(2)]
            for i, (dd, cc) in enumerate(pairs):
                nc.tensor.transpose(
                    out=lps[i // 4][:, (i % 4) * 128 : (i % 4 + 1) * 128],
                    in_=G_sb[:, dd, cc * 128 : (cc + 1) * 128],
                    identity=ident,
                )
            for i, (dd, cc) in enumerate(pairs):
                copy_evac(
                    i,
                    G_sb[:, cc, dd * 128 : (dd + 1) * 128],
                    lps[i // 4][:, (i % 4) * 128 : (i % 4 + 1) * 128],
                )
            st["G_sb"] = G_sb
            del st["G_ps"]

        def U_phase(b, st):
            U_sb = midp.tile([128, CT, C], ST, name="U_sb", tag="usb")
            for ic in range(CT):
                u_ps = ops.tile([128, C], F32, name="u_ps", tag="out")
                for e in range(CT):
                    nc.tensor.matmul(
                        out=u_ps,
                        lhsT=st["G_sb"][:, e, ic * 128 : (ic + 1) * 128],
                        rhs=wkt_sb[:, e, :],
                        start=(e == 0),
                        stop=(e == CT - 1),
                    )
                copy_evac(ic, U_sb[:, ic, :], u_ps)
            st["U_sb"] = U_sb

        def E_prep(b, st):
            st["attn"] = midp.tile([128, CT, C], ST, name="attn_sb", tag="attn")
            st["mx"] = vecp.tile([128, CT], F32, name="mx", tag="mx")
            st["negm"] = vecp.tile([128, CT], F32, name="negm", tag="negm")
            st["zsum"] = vecp.tile([128, CT], F32, name="zsum", tag="zsum")

        def E_cc(b, cc, st):
            # E block-row cc: 4 Gram matmuls + one K=2 rank-2 bias matmul;
            # softmax (max, exp+rowsum) reads the PSUM bank directly.  The
            # e_ps bank lives on the mm ring so out-phase matmuls on the
            # out ring never stall behind softmax reads.
            e_ps = mmps.tile([128, C], F32, name="e_ps", tag="mm")
            for i in range(CT):
                nc.tensor.matmul(
                    out=e_ps,
                    lhsT=wqt_sb[:, i, cc * 128 : (cc + 1) * 128],
                    rhs=st["U_sb"][:, i, :],
                    start=(i == 0),
                    stop=False,
                )
            nc.tensor.matmul(
                out=e_ps,
                lhsT=st["l2"][:, cc * 128 : (cc + 1) * 128],
                rhs=st["r2"],
                start=False,
                stop=True,
            )
            nc.vector.reduce_max(
                out=st["mx"][:, cc : cc + 1], in_=e_ps, axis=AX.X
            )
            nc.vector.tensor_scalar_mul(
                st["negm"][:, cc : cc + 1], st["mx"][:, cc : cc + 1], -1.0
            )
            nc.scalar.activation(
                out=st["attn"][:, cc, :],
                in_=e_ps,
                func=ACTF.Exp,
                bias=st["negm"][:, cc : cc + 1],
                scale=1.0,
                accum_out=st["zsum"][:, cc : cc + 1],
            )

        def finish_softmax(b, st):
            recip = vecp.tile([128, CT], F32, name="recip", tag="recip")
            nc.vector.reciprocal(out=recip, in_=st["zsum"])
            tts = vecp.tile([128, C], F32, name="tts", tag="tts", bufs=1)
            tcol = vecp.tile([128, CT], F32, name="tcol", tag="tcol")
            for cc in range(CT):
                nc.vector.tensor_mul(tts, st["attn"][:, cc, :], bv_rep)
                nc.vector.reduce_sum(out=tcol[:, cc : cc + 1], in_=tts, axis=AX.X)
            rt = vecp.tile([128, CT], F32, name="rt", tag="rt")
            nc.vector.tensor_mul(rt, tcol, recip)
            st["recip"] = recip
            st["rt"] = rt

        def attnT_half(b, st, dcs):
            if "attnT_sb" not in st:
                st["attnT_sb"] = midp.tile(
                    [128, CT, C], BF16, name="attnT_sb", tag="attnT"
                )
            attnT_sb = st["attnT_sb"]
            for dc in dcs:
                at_ps = mmps.tile([128, C], ST, name="at_ps", tag="mm")
                for t in range(CT):
                    nc.tensor.transpose(
                        out=at_ps[:, t * 128 : (t + 1) * 128],
                        in_=st["attn"][:, t, dc * 128 : (dc + 1) * 128],
                        identity=ident,
                    )
                copy_evac(dc, attnT_sb[:, dc, :], at_ps)

        def AT_half(b, st, ics):
            attnT_sb = st["attnT_sb"]
            if "AT" not in st:
                st["AT"] = midp.tile([128, CT, C], BF16, name="AT_sb", tag="atb")
            AT_sb = st["AT"]
            for ic in ics:
                a_ps = mmps.tile([128, C], F32, name="a_ps", tag="mm")
                for d_ in range(CT):
                    nc.tensor.matmul(
                        out=a_ps,
                        lhsT=wv_sb[:, d_, ic * 128 : (ic + 1) * 128],
                        rhs=attnT_sb[:, d_, :],
                        start=(d_ == 0),
                        stop=(d_ == CT - 1),
                    )
                copy_evac(ic + 1, AT_sb[:, ic, :], a_ps)

        def out_q_begin(b, q, st):
            st[f"stage{q}"] = outp.tile(
                [128, CT, QTR], BF16, name=f"stage_b{b}q{q}", tag="stage"
            )

        def out_cc(b, q, cc, st, use_gps=False):
            xb = st[f"xb{q}"]
            stage = st[f"stage{q}"]
            for pb in range(2):
                # late quarters rotate through the dead G-accumulator banks
                # too (5-deep ring) so the PSUM-evacuation affine latency
                # never gates the next matmul group
                g = st.get("ogrp", 0)
                st["ogrp"] = g + 1
                if use_gps and g % 5 == 3:
                    o_ps = gps.tile([128, 512], F32, name="o_ps_g0", tag="g0")
                elif use_gps and g % 5 == 4:
                    o_ps = gps.tile([128, 512], F32, name="o_ps_g1", tag="g1")
                else:
                    o_ps = ops.tile([128, 512], F32, name="o_ps", tag="out")
                for i in range(CT):
                    nc.tensor.matmul(
                        out=o_ps,
                        lhsT=st["AT"][:, i, cc * 128 : (cc + 1) * 128],
                        rhs=xb[:, i, pb * 512 : (pb + 1) * 512],
                        start=(i == 0),
                        stop=(i == CT - 1),
                    )
                if pb % 2 == 0:
                    nc.scalar.activation(
                        out=stage[:, cc, pb * 512 : (pb + 1) * 512],
                        in_=o_ps,
                        func=ACTF.Identity,
                        bias=st["rt"][:, cc : cc + 1],
                        scale=st["recip"][:, cc : cc + 1],
                    )
                else:
                    nc.vector.tensor_scalar(
                        out=stage[:, cc, pb * 512 : (pb + 1) * 512],
                        in0=o_ps,
                        scalar1=st["recip"][:, cc : cc + 1],
                        scalar2=st["rt"][:, cc : cc + 1],
                        op0=ALU.mult,
                        op1=ALU.add,
                    )

        def out_q_store(b, q, st, split=False):
            stage = st.pop(f"stage{q}")
            if not split:
                # one store per quarter on the gpsimd (SWDGE) queue: few
                # queue ops, never blocks the load queues
                nc.gpsimd.dma_start(
                    out=out_d[b, :, q * QTR : (q + 1) * QTR].rearrange(
                        "(t p) f -> p t f", p=128
                    ),
                    in_=stage,
                )
            else:
                # tail quarters: fan out per c-tile across idle queues so
                # the final drain is short
                for cc in range(CT):
                    eng = nc.sync if cc % 2 == 0 else nc.scalar
                    eng.dma_start(
                        out=out_d[
                            b, cc * 128 : (cc + 1) * 128, q * QTR : (q + 1) * QTR
                        ],
                        in_=stage[:, cc, :],
                    )

        def out_phase(b, q, st, split=False, use_gps=False):
            out_q_begin(b, q, st)
            for cc in range(CT):
                out_cc(b, q, cc, st, use_gps=use_gps)
            out_q_store(b, q, st, split=split)

        # ---- schedule: batch-1 G matmuls are threaded into batch-0's
        # softmax/evac latency windows (and b0's last out quarter into
        # b1's) so the PE never idles long enough to re-throttle ----
        for q in range(NQ):
            G_ptiles(0, st0, q, range(QT_Q))
        G_copy(0, st0)
        wv_sb = load_w("wv_sb", wv_d, BF16)
        load_xt(1, 0, st1)
        U_phase(0, st0)
        E_prep(0, st0)
        E_cc(0, 0, st0)
        E_cc(0, 1, st0)
        load_xt(1, 1, st1)
        G_ptiles(1, st1, 0, range(0, 4))
        E_cc(0, 2, st0)
        G_ptiles(1, st1, 0, range(4, 8))
        E_cc(0, 3, st0)
        load_xb(0, 0, st0)
        G_ptiles(1, st1, 1, range(0, 4))
        finish_softmax(0, st0)
        attnT_half(0, st0, [0, 1, 2, 3])
        AT_half(0, st0, [0, 1, 2, 3])
        G_ptiles(1, st1, 1, range(4, 8))
        load_xb(0, 1, st0)
        load_xt(1, 2, st1)
        out_phase(0, 0, st0)
        G_ptiles(1, st1, 2, range(0, 8))
        load_xt(1, 3, st1)
        load_xb(0, 2, st0)
        out_phase(0, 1, st0)
        G_ptiles(1, st1, 3, range(0, 8))
        load_xb(0, 3, st0)
        out_q_begin(0, 2, st0)
        G_copy(1, st1)
        out_cc(0, 2, 0, st0, use_gps=True)
        load_xb(1, 0, st1)
        U_phase(1, st1)
        E_prep(1, st1)
        E_cc(1, 0, st1)
        out_cc(0, 2, 1, st0, use_gps=True)
        E_cc(1, 1, st1)
        out_cc(0, 2, 2, st0, use_gps=True)
        load_xb(1, 1, st1)
        E_cc(1, 2, st1)
        out_cc(0, 2, 3, st0, use_gps=True)
        out_q_store(0, 2, st0)
        E_cc(1, 3, st1)
        out_q_begin(0, 3, st0)
        out_cc(0, 3, 0, st0, use_gps=True)
        finish_softmax(1, st1)
        load_xb(1, 2, st1)
        attnT_half(1, st1, [0, 1])
        out_cc(0, 3, 1, st0, use_gps=True)
        attnT_half(1, st1, [2, 3])
        out_cc(0, 3, 2, st0, use_gps=True)
        AT_half(1, st1, [0, 1])
        out_cc(0, 3, 3, st0, use_gps=True)
        out_q_store(0, 3, st0)
        AT_half(1, st1, [2, 3])
        load_xb(1, 3, st1)
        out_phase(1, 0, st1, use_gps=True)
        out_phase(1, 1, st1, use_gps=True)
        out_phase(1, 2, st1, split=True, use_gps=True)
        out_phase(1, 3, st1, split=True, use_gps=True)

    nc.compile()
    return nc


_CACHE = {}


def _get_nc():
    if "nc" not in _CACHE:
        _CACHE["nc"] = build_nc()
    return _CACHE["nc"]


def make_in_maps(x, Wq, bq, Wk, bk, Wv, bv):
    x = np.asarray(x, np.float32)
    Wq = np.asarray(Wq, np.float32)
    Wk = np.asarray(Wk, np.float32)
    Wv = np.asarray(Wv, np.float32)
    bq = np.asarray(bq, np.float32)
    bk = np.asarray(bk, np.float32)
    bv = np.asarray(bv, np.float32)
    f16 = np.float16
    shared = {
        "wqt": np.ascontiguousarray(Wq.T.astype(f16)),
        "wkt": np.ascontiguousarray(Wk.T.astype(f16)),
        "wv": np.ascontiguousarray(Wv.astype(ml_dtypes.bfloat16)),
        "bv_row": np.ascontiguousarray(bv[None, :]),
        "ident": np.eye(128, dtype=f16),
    }
    maps = []
    for i in range(N_CORES):
        xs = x[BPC * i : BPC * (i + 1)]                    # [BPC, C, P]
        s = xs.sum(axis=2, dtype=np.float64).astype(np.float32)
        qs = s @ Wq.T
        r = s @ Wk.T + np.float32(P) * bk[None, :]
        l2 = np.stack([qs, np.broadcast_to(bq, (BPC, C))], axis=1)
        r2 = np.stack([np.broadcast_to(bk, (BPC, C)), r], axis=1)
        maps.append(
            {
                "xt": np.ascontiguousarray(
                    xs.transpose(0, 2, 1)
                    .reshape(BPC, P // 256, 2, 128, C)
                    .transpose(0, 1, 3, 2, 4)
                    .reshape(BPC, P // 256, 128, 2 * C)
                    .astype(f16)
                ),
                "xb": np.ascontiguousarray(xs.astype(ml_dtypes.bfloat16)),
                "l2": np.ascontiguousarray(l2.astype(f16)),
                "r2": np.ascontiguousarray(r2.astype(f16)),
                **shared,
            }
        )
    return maps


def run(inputs, trace=False, tmpdir=None):
    nc = _get_nc()
    in_maps = make_in_maps(**inputs)
    res = run_bass_kernel_spmd(
        nc, in_maps, core_ids=list(range(N_CORES)), trace=trace, tmpdir=tmpdir
    )
    out = np.concatenate(
        [res.results[i]["out"].astype(np.float32) for i in range(N_CORES)], axis=0
    )
    return out, res


def kernel(**inputs) -> np.ndarray:
    out, _ = run(inputs, trace=False)
    return out
